# revision 49
# baseline (speedup 1.0000x reference)
"""Causal self-attention (B=2, T=2048, C=1024, H=16) on 8 trn2 NeuronCores.

Sharding: core i handles batch b = i // 4 and head-group hg = i % 4
(4 heads each). Data-parallel over B, tensor-parallel over heads:
each core computes q/k/v for its 4 heads, full causal attention locally,
and a partial projection out = y_heads @ W_proj[rows]; the host sums the
4 partials per batch. No collectives.

All compute in bf16 (inputs converted host-side; PSUM accumulates fp32).

Layout: transposed space, no on-chip transposes:
  - host passes xT = x[b].T  [C, T] bf16
  - qT/kT [d, T] straight out of the qkv matmul; per m, qt[m]/kt[m]
    [128, T] stack head 2m in partitions 0:64 and head 2m+1 in 64:128
  - scores: per k-tile, TWO concurrent K=64 matmuls via PE row tiling
    (head A in array rows 0-63, head B in rows 64-127; auto
    tile_position from base partitions) into one [128, 1024] PSUM tile
    [A | B] -> ONE exp per k-tile on ACT (bf16 out)
  - mask: multiply diagonal 128x128 bands by 0/1 mask post-exp (DVE)
  - y/denom: psy[65, 512] accumulates va_aug.T @ pt over k-tiles
    (va col 64 = ones)
  - divide: one [65,512] copy out of PSUM, reciprocal, gpsimd
    partition_broadcast, multiply into yt (bf16)
  - proj: yt as stationary bf16 (FWL), out partial [T, C] fp32,
    PSUM->SBUF copy on the Pool engine, DMA out
Schedule: fine-grained interleave of qkv/proj units into attention
k-tile yield points so PE and ACT stay busy together.
"""

import sys

import numpy as np

sys.path.insert(0, "/opt/trn_rl_repo")

B, T, C = 2, 2048, 1024
N_HEAD = 16
D = C // N_HEAD          # 64
HPC = N_HEAD // 4        # 4 heads per core
CS = HPC * D             # 256 = per-core slice width of q/k/v
NCHUNK = C // 128        # 8 contraction chunks over C
NT = T // 128            # 16 row tiles
NQ = T // 512            # 4 query tiles of 512
SCALE = 1.0 / np.sqrt(D)

_CACHE = {}


def _build():
    from collections import deque

    import concourse.bacc as bacc
    import concourse.mybir as mybir
    import concourse.tile as tile

    F32 = mybir.dt.float32
    BF16 = mybir.dt.bfloat16

    nc = bacc.Bacc("TRN2", target_bir_lowering=False, debug=False, num_devices=8)

    xT = nc.dram_tensor("xT", [C, T], BF16, kind="ExternalInput").ap()
    wq = nc.dram_tensor("wq", [128, NCHUNK * CS], BF16, kind="ExternalInput").ap()
    wk = nc.dram_tensor("wk", [128, NCHUNK * CS], BF16, kind="ExternalInput").ap()
    wv = nc.dram_tensor("wv", [128, NCHUNK * CS], BF16, kind="ExternalInput").ap()
    wp = nc.dram_tensor("wp", [128, 2 * C], BF16, kind="ExternalInput").ap()
    mask = nc.dram_tensor("mask", [128, 256], BF16, kind="ExternalInput").ap()
    out = nc.dram_tensor("out", [T, C], BF16, kind="ExternalOutput").ap()

    with tile.TileContext(nc) as tc:
        with (
            tc.tile_pool(name="persist", bufs=1) as pp,
            tc.tile_pool(name="consts", bufs=1) as cp,
            tc.tile_pool(name="xw", bufs=1) as xw,
            tc.tile_pool(name="xs", bufs=2) as xsp,
            tc.tile_pool(name="pt", bufs=6) as ptp,
            tc.tile_pool(name="sm", bufs=2) as smp,
            tc.tile_pool(name="po", bufs=4) as pop,
            tc.tile_pool(name="psm", bufs=2, space="PSUM") as psm_p,
            tc.tile_pool(name="psb", bufs=2, space="PSUM") as psb_p,
            tc.tile_pool(name="psy", bufs=2, space="PSUM") as psy_p,
        ):
            # ---------------- persistent SBUF ----------------
            # qt/kt[m]: head 2m in partitions 0:64, head 2m+1 in 64:128
            qt = [pp.tile([128, T], BF16, name=f"qt{m}", tag=f"qt{m}")
                  for m in range(2)]
            kt = [pp.tile([128, T], BF16, name=f"kt{m}", tag=f"kt{m}")
                  for m in range(2)]
            yt = [pp.tile([128, T], BF16, name=f"yt{m}", tag=f"yt{m}")
                  for m in range(2)]
            # v_aug per row-tile: [128, 4 heads, 65] (col 64 = ones)
            va = [pp.tile([128, HPC, D + 1], BF16, name=f"va{t}", tag=f"va{t}")
                  for t in range(NT)]
            mk = cp.tile([128, 2, 128], BF16, tag="mask")
            wpt = cp.tile([128, 2, C], BF16, tag="wp")
            ones_bf = cp.tile([128, HPC], BF16, tag="ones")
            warm_src = cp.tile([128, 1], F32, tag="warmsrc")
            warm = cp.tile([128, 1], F32, tag="warm")
            zsrc = cp.tile([128, 512], BF16, tag="zsrc")

            # ---------------- DMAs, critical-path first ----------------
            # weights as single DMAs (each Sync trigger costs ~0.7us, so
            # fewer, bigger transfers win); x per-chunk tiles so the first
            # matmul only waits for its own chunk's DMA
            wqt = xw.tile([128, NCHUNK, CS], BF16, tag="wq")
            wkt = xw.tile([128, NCHUNK, CS], BF16, tag="wk")
            wvt = xw.tile([128, NCHUNK, CS], BF16, tag="wv")
            # 4 slices share 2 slots per chunk: slice ns+2's DMA waits until
            # slice ns is consumed (automatic WAR dep via the shared tag)
            xts = [[xsp.tile([128, 512], BF16, name=f"xt{ns}_{c}",
                             tag=f"xt{c}") for c in range(NCHUNK)]
                   for ns in range(NQ)]

            def dma_x(ns, split=False):
                # split=True: odd chunks via the Scalar HWDGE ring so the two
                # rings stream in parallel (~150GB/s each) at startup
                for c in range(NCHUNK):
                    eng = nc.scalar if (split and c % 2) else nc.sync
                    eng.dma_start(
                        xts[ns][c][:],
                        xT[c * 128:(c + 1) * 128, ns * 512:(ns + 1) * 512],
                    )

            # prewarm: spin the PE on zeros while the first DMAs land, so the
            # HAM clock gate is already at 8/8 when real matmuls start
            nc.vector.memset(zsrc[:], 0.0)
            for i in range(5):
                pw = psb_p.tile([128, 512], F32, tag="psb", name="pw")
                nc.tensor.matmul(pw[:], zsrc[:, 0:128], zsrc[:],
                                 start=True, stop=True)

            # mask first: every k-tile of attention(0,*) is diagonal and its
            # y stalls on the mask multiply
            nc.sync.dma_start(mk[:].rearrange("p a b -> p (a b)"), mask[:])
            nc.scalar.dma_start(wkt[:].rearrange("p c n -> p (c n)"), wk[:])
            nc.sync.dma_start(wqt[:].rearrange("p c n -> p (c n)"), wq[:])
            dma_x(0, split=True)
            nc.scalar.dma_start(wvt[:].rearrange("p c n -> p (c n)"), wv[:])
            dma_x(1, split=True)
            dma_x(2)
            nc.sync.dma_start(wpt[:].rearrange("p c n -> p (c n)"), wp[:])
            dma_x(3)

            nc.gpsimd.memset(ones_bf[:], 1.0)
            nc.gpsimd.memset(warm_src[:], 1.0)
            # warm the ACT exp table early (off the critical path)
            nc.scalar.activation(warm[:], warm_src[:],
                                 mybir.ActivationFunctionType.Exp, scale=1.0)

            # ---------------- qkv units ----------------
            def qk_unit(ns, m, which):
                sl = slice(ns * 512, (ns + 1) * 512)
                w_all = wqt if which == "q" else wkt
                ps = psb_p.tile([128, 512], F32, tag="psb", name="psqk")
                for c in range(NCHUNK):
                    nc.tensor.matmul(
                        ps[:],
                        w_all[:, c, m * 128:(m + 1) * 128],
                        xts[ns][c][:],
                        start=(c == 0),
                        stop=(c == NCHUNK - 1),
                    )
                # kt copies gate the PE's next scores LDWEIGHTS — keep them
                # on ACT (near-idle in qkv stretches); q feeds the matmul rhs
                # later, DVE latency is fine and it unloads the exp engine
                if which == "q":
                    nc.vector.tensor_copy(qt[m][:, sl], ps[:])
                else:
                    nc.scalar.copy(kt[m][:, sl], ps[:])

            def v_unit(ns, t):
                ps = psb_p.tile([128, CS], F32, tag="psb", name="psv")
                for c in range(NCHUNK):
                    nc.tensor.matmul(
                        ps[:],
                        xts[ns][c][:, (t % 4) * 128:(t % 4 + 1) * 128],
                        wvt[:, c, :],
                        start=(c == 0),
                        stop=(c == NCHUNK - 1),
                    )
                nc.scalar.copy(
                    va[t][:, :, 0:D],
                    ps[:].rearrange("p (h d) -> p h d", h=HPC),
                )
                nc.vector.tensor_copy(va[t][:, :, D], ones_bf[:])

            # ---------------- attention ----------------
            def divide(h, j, psy, s0=0, s1=512):
                # copy y and den out fast (releases the psy bank); the slow
                # recip/broadcast chain then runs off the critical path.
                # reciprocal_approx_fast requires a partition-0 input on HW.
                hq, ho = h // 2, (h % 2) * 64
                w = s1 - s0
                yu = smp.tile([D, 512], F32, tag="yu", name="yu")
                nc.vector.tensor_copy(yu[:, 0:w], psy[0:D, s0:s1])
                den = smp.tile([1, 512], F32, tag="den", name="den")
                nc.vector.tensor_copy(den[:, 0:w], psy[D:D + 1, s0:s1])
                rec = smp.tile([1, 512], F32, tag="rec", name="rec")
                nc.vector.reciprocal_approx_fast(rec[:, 0:w], den[:, 0:w])
                bc = smp.tile([D, 512], F32, tag="bc", name="bc")
                nc.gpsimd.partition_broadcast(bc[:, 0:w], rec[:, 0:w])
                nc.vector.tensor_mul(
                    yt[hq][ho:ho + 64, j * 512 + s0:j * 512 + s1],
                    yu[:, 0:w],
                    bc[:, 0:w],
                )

            def attention(j, hp, fine_tail=False, pre_tail=None):
                nkb = 4 * (j + 1)
                psyA = psy_p.tile([D + 1, 512], F32, tag="psy", name="psyA")
                psyB = psy_p.tile([D + 1, 512], F32, tag="psy", name="psyB")
                q0_sl = j * 512
                pending = {}

                def y_acc(kb):
                    pq0, ppt = pending.pop(kb)
                    nc.tensor.matmul(
                        psyA[:, pq0:512], va[kb][:, 2 * hp, :],
                        ppt[:, 0, pq0:512],
                        start=(kb == 0), stop=(kb == nkb - 1),
                    )
                    nc.tensor.matmul(
                        psyB[:, pq0:512], va[kb][:, 2 * hp + 1, :],
                        ppt[:, 1, pq0:512],
                        start=(kb == 0), stop=(kb == nkb - 1),
                    )

                for kb in range(nkb):
                    di = kb - 4 * j
                    q0 = 128 * di if di > 0 else 0
                    ksl = slice(kb * 128, (kb + 1) * 128)
                    qsl = slice(q0_sl + q0, q0_sl + 512)
                    # filler BEFORE scores: if scores must wait for the exp
                    # pipeline (psm WAR), the filler runs during the wait
                    # instead of head-blocking behind it
                    yield
                    pss = psm_p.tile([128, 2, 512], F32, tag="psm", name="pss")
                    # two K=64 matmuls, concurrent via PE row tiling
                    nc.tensor.matmul(
                        pss[:, 0, q0:512], kt[hp][0:64, ksl], qt[hp][0:64, qsl],
                        start=True, stop=True,
                    )
                    nc.tensor.matmul(
                        pss[:, 1, q0:512], kt[hp][64:128, ksl],
                        qt[hp][64:128, qsl],
                        start=True, stop=True,
                    )
                    pt = ptp.tile([128, 2, 512], BF16, tag="pt", name="pt")
                    # one exp for both heads: strided AP over the written spans
                    nc.scalar.activation(
                        pt[:, :, q0:512], pss[:, :, q0:512],
                        mybir.ActivationFunctionType.Exp, scale=float(SCALE),
                    )
                    if di >= 0:
                        # zero the upper triangle of the diagonal band, both
                        # heads in one strided op.
                        # NB: must NOT run on gpsimd — mixing native tensor
                        # ops with partition_broadcast there forces a ~5us
                        # microcode library swap per alternation.
                        nc.vector.tensor_mul(
                            pt[:, :, q0:q0 + 128], pt[:, :, q0:q0 + 128],
                            mk[:],
                        )
                    pending[kb] = (q0, pt)
                    yield
                    # y lags two k-tiles so exp+mask latency is fully hidden
                    if kb >= 2:
                        y_acc(kb - 2)
                if nkb >= 2:
                    y_acc(nkb - 2)
                y_acc(nkb - 1)
                if not fine_tail:
                    divide(2 * hp, j, psyA)
                    divide(2 * hp + 1, j, psyB)
                else:
                    # last task: pipeline per-256-col divide chunks straight
                    # into their proj row-tiles so the tail drains overlapped
                    # (128-wide chunks make the gpsimd broadcasts, ~850ns
                    # fixed cost each, dominate)
                    if pre_tail is not None:
                        pre_tail()
                    for s in range(2):
                        divide(2 * hp, j, psyA, s * 256, (s + 1) * 256)
                        divide(2 * hp + 1, j, psyB, s * 256, (s + 1) * 256)
                        proj1_unit(j, 4 * j + 2 * s)
                        proj1_unit(j, 4 * j + 2 * s + 1)

            # cc-split projection for the final j-block: the yt[0] half is
            # computed as filler during the last attention task; the tail
            # only runs the yt[1] matmul and an add
            po0 = {}

            def proj0_unit(t, nb):
                ps = psb_p.tile([128, 512], F32, tag="psb", name="ps0")
                nc.tensor.matmul(
                    ps[:], yt[0][:, t * 128:(t + 1) * 128],
                    wpt[:, 0, nb * 512:(nb + 1) * 512],
                    start=True, stop=True,
                )
                p0 = pp.tile([128, 512], F32, name=f"po0_{t}_{nb}",
                             tag=f"po0_{t}_{nb}")
                nc.vector.tensor_copy(p0[:], ps[:])
                po0[(t, nb)] = p0

            def proj1_unit(j, t):
                for nb in range(2):
                    ps = psb_p.tile([128, 512], F32, tag="psb", name="ps1")
                    nc.tensor.matmul(
                        ps[:], yt[1][:, t * 128:(t + 1) * 128],
                        wpt[:, 1, nb * 512:(nb + 1) * 512],
                        start=True, stop=True,
                    )
                    ot = pop.tile([128, 512], BF16, tag="po", name="po")
                    nc.vector.tensor_add(ot[:], ps[:], po0[(t, nb)][:])
                    nc.sync.dma_start(
                        out[t * 128:(t + 1) * 128,
                            nb * 512:(nb + 1) * 512],
                        ot[:],
                    )

            def proj_unit(j, t, tail=False):
                for nb in range(2):
                    ps = psb_p.tile([128, 512], F32, tag="psb", name="pso")
                    for cc in range(2):
                        nc.tensor.matmul(
                            ps[:],
                            yt[cc][:, t * 128:(t + 1) * 128],
                            wpt[:, cc, nb * 512:(nb + 1) * 512],
                            start=(cc == 0),
                            stop=(cc == 1),
                        )
                    ot = pop.tile([128, 512], BF16, tag="po", name="po")
                    if tail:
                        nc.scalar.copy(ot[:], ps[:])  # ACT idle in the tail
                    else:
                        nc.vector.tensor_copy(ot[:], ps[:])
                    nc.sync.dma_start(
                        out[t * 128:(t + 1) * 128,
                            nb * 512:(nb + 1) * 512],
                        ot[:],
                    )

            # ---------------- interleaved schedule ----------------
            def b_units(ns, q_first=False):
                units = []
                if q_first:
                    # b0: wk/wv DMAs land after wq+x0 — do both q units first
                    for m in range(2):
                        units.append((f"q{m}", lambda ns=ns, m=m: qk_unit(ns, m, "q")))
                    for m in range(2):
                        units.append((f"k{m}", lambda ns=ns, m=m: qk_unit(ns, m, "k")))
                else:
                    for m in range(2):
                        units.append((f"q{m}", lambda ns=ns, m=m: qk_unit(ns, m, "q")))
                        units.append((f"k{m}", lambda ns=ns, m=m: qk_unit(ns, m, "k")))
                for t in range(4 * ns, 4 * ns + 4):
                    units.append((f"v{t}", lambda ns=ns, t=t: v_unit(ns, t)))
                return units

            # emit only q0/k0 of segment 0 eagerly so attention(0,0) can
            # start feeding ACT immediately; the rest flows via fillers
            b0 = b_units(0, q_first=True)
            bq = deque()
            emitted = set()
            for name, u in b0:
                if name in ("q0", "k0"):
                    u()
                    emitted.add((0, name))
                else:
                    bq.append((0, name, u))
            for ns in range(1, NQ):
                for name, u in b_units(ns):
                    bq.append((ns, name, u))
            pq = deque()            # proj units, unlocked per j-block
            nyield = [0]

            def emit_filler(allow_proj=True):
                if bq:
                    ns, name, u = bq.popleft()
                    u()
                    emitted.add((ns, name))
                elif pq and allow_proj:
                    pq.popleft()()

            def drain_pq():
                while bq or pq:
                    emit_filler()

            tasks = [(j, hp) for j in range(NQ) for hp in range(2)]
            last = tasks[-1]
            for j, hp in tasks:
                # att(j,hp) needs only q/k of segment j for its own head
                # pair before starting (k of earlier segments already in);
                # v units are consumed by lagged y's and drain via fillers
                while (j, f"q{hp}") not in emitted or (j, f"k{hp}") not in emitted:
                    emit_filler()
                for _ in attention(j, hp, fine_tail=((j, hp) == last),
                                   pre_tail=drain_pq):
                    nyield[0] += 1
                    # hold proj fillers for the ACT-bound late phase (j>=2),
                    # paced at one per k-tile (every 2nd yield)
                    emit_filler(allow_proj=(j >= 2 and nyield[0] % 2 == 0))
                if hp == 1 and j < NQ - 1:
                    for t in range(4 * j, 4 * j + 4):
                        pq.append(lambda j=j, t=t: proj_unit(j, t))
                if (j, hp) == (last[0], 0):
                    # unlock the cc0 half of the final projection
                    for t in range(4 * last[0], 4 * last[0] + 4):
                        for nb in range(2):
                            pq.append(lambda t=t, nb=nb: proj0_unit(t, nb))
            drain_pq()

    nc.compile()
    return nc


def _causal_mask():
    kk = np.arange(128)[:, None]
    cc = np.arange(128)[None, :]
    return (cc >= kk).astype(np.float32)


def _get_nc():
    if "nc" not in _CACHE:
        _CACHE["nc"] = _build()
    return _CACHE["nc"]


def _run(x, W_qkv, W_proj, trace=False, trace_cores=None):
    import ml_dtypes
    from concourse.bass_utils import run_bass_kernel_spmd

    BF = ml_dtypes.bfloat16
    x = np.asarray(x, dtype=np.float32)
    W_qkv = np.asarray(W_qkv, dtype=np.float32)
    W_proj = np.asarray(W_proj, dtype=np.float32)

    nc = _get_nc()
    m1 = _causal_mask()
    mask = np.ascontiguousarray(np.concatenate([m1, m1], axis=1).astype(BF))
    in_maps = []
    for core in range(8):
        b, hg = core // 4, core % 4
        sl = slice(hg * CS, (hg + 1) * CS)

        def warr(w):  # [K, N] -> [128, (K//128)*N] chunk-major per partition
            return np.ascontiguousarray(
                w.reshape(w.shape[0] // 128, 128, -1)
                .transpose(1, 0, 2).reshape(128, -1).astype(BF)
            )

        in_maps.append({
            "xT": np.ascontiguousarray(x[b].T.astype(BF)),
            "wq": warr(W_qkv[:, sl]),
            "wk": warr(W_qkv[:, C + hg * CS:C + (hg + 1) * CS]),
            "wv": warr(W_qkv[:, 2 * C + hg * CS:2 * C + (hg + 1) * CS]),
            "wp": warr(W_proj[sl, :]),
            "mask": mask,
        })

    res = run_bass_kernel_spmd(
        nc, in_maps, list(range(8)), trace=trace, trace_cores=trace_cores
    )
    outp = np.zeros((B, T, C), dtype=np.float32)
    for core in range(8):
        outp[core // 4] += res.results[core]["out"].astype(np.float32)
    return outp, res


def kernel(x, W_qkv, W_proj):
    outp, _ = _run(x, W_qkv, W_proj)
    return outp


# revision 50
# speedup vs baseline: 1.0036x; 1.0036x over previous
"""Causal self-attention (B=2, T=2048, C=1024, H=16) on 8 trn2 NeuronCores.

Sharding: core i handles batch b = i // 4 and head-group hg = i % 4
(4 heads each). Data-parallel over B, tensor-parallel over heads:
each core computes q/k/v for its 4 heads, full causal attention locally,
and a partial projection out = y_heads @ W_proj[rows]; the host sums the
4 partials per batch. No collectives.

All compute in bf16 (inputs converted host-side; PSUM accumulates fp32).

Layout: transposed space, no on-chip transposes:
  - host passes xT = x[b].T  [C, T] bf16
  - qT/kT [d, T] straight out of the qkv matmul; per m, qt[m]/kt[m]
    [128, T] stack head 2m in partitions 0:64 and head 2m+1 in 64:128
  - scores: per k-tile, TWO concurrent K=64 matmuls via PE row tiling
    (head A in array rows 0-63, head B in rows 64-127; auto
    tile_position from base partitions) into one [128, 1024] PSUM tile
    [A | B] -> ONE exp per k-tile on ACT (bf16 out)
  - mask: multiply diagonal 128x128 bands by 0/1 mask post-exp (DVE)
  - y/denom: psy[65, 512] accumulates va_aug.T @ pt over k-tiles
    (va col 64 = ones)
  - divide: one [65,512] copy out of PSUM, reciprocal, gpsimd
    partition_broadcast, multiply into yt (bf16)
  - proj: yt as stationary bf16 (FWL), out partial [T, C] fp32,
    PSUM->SBUF copy on the Pool engine, DMA out
Schedule: fine-grained interleave of qkv/proj units into attention
k-tile yield points so PE and ACT stay busy together.
"""

import sys

import numpy as np

sys.path.insert(0, "/opt/trn_rl_repo")

B, T, C = 2, 2048, 1024
N_HEAD = 16
D = C // N_HEAD          # 64
HPC = N_HEAD // 4        # 4 heads per core
CS = HPC * D             # 256 = per-core slice width of q/k/v
NCHUNK = C // 128        # 8 contraction chunks over C
NT = T // 128            # 16 row tiles
NQ = T // 512            # 4 query tiles of 512
SCALE = 1.0 / np.sqrt(D)

_CACHE = {}


def _build():
    from collections import deque

    import concourse.bacc as bacc
    import concourse.mybir as mybir
    import concourse.tile as tile

    F32 = mybir.dt.float32
    BF16 = mybir.dt.bfloat16

    nc = bacc.Bacc("TRN2", target_bir_lowering=False, debug=False, num_devices=8)

    xT = nc.dram_tensor("xT", [C, T], BF16, kind="ExternalInput").ap()
    wq = nc.dram_tensor("wq", [128, NCHUNK * CS], BF16, kind="ExternalInput").ap()
    wk = nc.dram_tensor("wk", [128, NCHUNK * CS], BF16, kind="ExternalInput").ap()
    wv = nc.dram_tensor("wv", [128, NCHUNK * CS], BF16, kind="ExternalInput").ap()
    wp = nc.dram_tensor("wp", [128, 2 * C], BF16, kind="ExternalInput").ap()
    mask = nc.dram_tensor("mask", [128, 256], BF16, kind="ExternalInput").ap()
    out = nc.dram_tensor("out", [T, C], BF16, kind="ExternalOutput").ap()

    with tile.TileContext(nc) as tc:
        with (
            tc.tile_pool(name="persist", bufs=1) as pp,
            tc.tile_pool(name="consts", bufs=1) as cp,
            tc.tile_pool(name="xw", bufs=1) as xw,
            tc.tile_pool(name="xs", bufs=2) as xsp,
            tc.tile_pool(name="pt", bufs=6) as ptp,
            tc.tile_pool(name="sm", bufs=2) as smp,
            tc.tile_pool(name="po", bufs=4) as pop,
            tc.tile_pool(name="psm", bufs=2, space="PSUM") as psm_p,
            tc.tile_pool(name="psb", bufs=2, space="PSUM") as psb_p,
            tc.tile_pool(name="psy", bufs=2, space="PSUM") as psy_p,
        ):
            # ---------------- persistent SBUF ----------------
            # qt/kt[m]: head 2m in partitions 0:64, head 2m+1 in 64:128
            qt = [pp.tile([128, T], BF16, name=f"qt{m}", tag=f"qt{m}")
                  for m in range(2)]
            kt = [pp.tile([128, T], BF16, name=f"kt{m}", tag=f"kt{m}")
                  for m in range(2)]
            yt = [pp.tile([128, T], BF16, name=f"yt{m}", tag=f"yt{m}")
                  for m in range(2)]
            # v_aug per row-tile: [128, 4 heads, 65] (col 64 = ones)
            va = [pp.tile([128, HPC, D + 1], BF16, name=f"va{t}", tag=f"va{t}")
                  for t in range(NT)]
            mk = cp.tile([128, 2, 128], BF16, tag="mask")
            wpt = cp.tile([128, 2, C], BF16, tag="wp")
            ones_bf = cp.tile([128, HPC], BF16, tag="ones")
            warm_src = cp.tile([128, 1], F32, tag="warmsrc")
            warm = cp.tile([128, 1], F32, tag="warm")
            zsrc = cp.tile([128, 512], BF16, tag="zsrc")

            # ---------------- DMAs, critical-path first ----------------
            # weights as single DMAs (each Sync trigger costs ~0.7us, so
            # fewer, bigger transfers win); x per-chunk tiles so the first
            # matmul only waits for its own chunk's DMA
            wqt = xw.tile([128, NCHUNK, CS], BF16, tag="wq")
            wkt = xw.tile([128, NCHUNK, CS], BF16, tag="wk")
            wvt = xw.tile([128, NCHUNK, CS], BF16, tag="wv")
            # 4 slices share 2 slots per chunk: slice ns+2's DMA waits until
            # slice ns is consumed (automatic WAR dep via the shared tag)
            xts = [[xsp.tile([128, 512], BF16, name=f"xt{ns}_{c}",
                             tag=f"xt{c}") for c in range(NCHUNK)]
                   for ns in range(NQ)]

            def dma_x(ns, split=False):
                # split=True: odd chunks via the Scalar HWDGE ring so the two
                # rings stream in parallel (~150GB/s each) at startup
                for c in range(NCHUNK):
                    eng = nc.scalar if (split and c % 2) else nc.sync
                    eng.dma_start(
                        xts[ns][c][:],
                        xT[c * 128:(c + 1) * 128, ns * 512:(ns + 1) * 512],
                    )

            # prewarm: spin the PE on zeros while the first DMAs land, so the
            # HAM clock gate is already at 8/8 when real matmuls start
            nc.vector.memset(zsrc[:], 0.0)
            for i in range(12):
                pw = psb_p.tile([128, 512], F32, tag="psb", name="pw")
                nc.tensor.matmul(pw[:], zsrc[:, 0:128], zsrc[:],
                                 start=True, stop=True)

            # mask first: every k-tile of attention(0,*) is diagonal and its
            # y stalls on the mask multiply
            nc.sync.dma_start(mk[:].rearrange("p a b -> p (a b)"), mask[:])
            nc.scalar.dma_start(wkt[:].rearrange("p c n -> p (c n)"), wk[:])
            nc.sync.dma_start(wqt[:].rearrange("p c n -> p (c n)"), wq[:])
            dma_x(0, split=True)
            nc.scalar.dma_start(wvt[:].rearrange("p c n -> p (c n)"), wv[:])
            dma_x(1, split=True)
            dma_x(2)
            nc.sync.dma_start(wpt[:].rearrange("p c n -> p (c n)"), wp[:])
            dma_x(3)

            nc.gpsimd.memset(ones_bf[:], 1.0)
            nc.gpsimd.memset(warm_src[:], 1.0)
            # warm the ACT exp table early (off the critical path)
            nc.scalar.activation(warm[:], warm_src[:],
                                 mybir.ActivationFunctionType.Exp, scale=1.0)

            # ---------------- qkv units ----------------
            def qk_unit(ns, m, which):
                sl = slice(ns * 512, (ns + 1) * 512)
                w_all = wqt if which == "q" else wkt
                ps = psb_p.tile([128, 512], F32, tag="psb", name="psqk")
                for c in range(NCHUNK):
                    nc.tensor.matmul(
                        ps[:],
                        w_all[:, c, m * 128:(m + 1) * 128],
                        xts[ns][c][:],
                        start=(c == 0),
                        stop=(c == NCHUNK - 1),
                    )
                # kt copies gate the PE's next scores LDWEIGHTS — keep them
                # on ACT (near-idle in qkv stretches); q feeds the matmul rhs
                # later, DVE latency is fine and it unloads the exp engine
                if which == "q":
                    nc.vector.tensor_copy(qt[m][:, sl], ps[:])
                else:
                    nc.scalar.copy(kt[m][:, sl], ps[:])

            def v_unit(ns, t):
                ps = psb_p.tile([128, CS], F32, tag="psb", name="psv")
                for c in range(NCHUNK):
                    nc.tensor.matmul(
                        ps[:],
                        xts[ns][c][:, (t % 4) * 128:(t % 4 + 1) * 128],
                        wvt[:, c, :],
                        start=(c == 0),
                        stop=(c == NCHUNK - 1),
                    )
                nc.scalar.copy(
                    va[t][:, :, 0:D],
                    ps[:].rearrange("p (h d) -> p h d", h=HPC),
                )
                nc.vector.tensor_copy(va[t][:, :, D], ones_bf[:])

            # ---------------- attention ----------------
            def divide(h, j, psy, s0=0, s1=512):
                # copy y and den out fast (releases the psy bank); the slow
                # recip/broadcast chain then runs off the critical path.
                # reciprocal_approx_fast requires a partition-0 input on HW.
                hq, ho = h // 2, (h % 2) * 64
                w = s1 - s0
                yu = smp.tile([D, 512], F32, tag="yu", name="yu")
                nc.vector.tensor_copy(yu[:, 0:w], psy[0:D, s0:s1])
                den = smp.tile([1, 512], F32, tag="den", name="den")
                nc.vector.tensor_copy(den[:, 0:w], psy[D:D + 1, s0:s1])
                rec = smp.tile([1, 512], F32, tag="rec", name="rec")
                nc.vector.reciprocal_approx_fast(rec[:, 0:w], den[:, 0:w])
                bc = smp.tile([D, 512], F32, tag="bc", name="bc")
                nc.gpsimd.partition_broadcast(bc[:, 0:w], rec[:, 0:w])
                nc.vector.tensor_mul(
                    yt[hq][ho:ho + 64, j * 512 + s0:j * 512 + s1],
                    yu[:, 0:w],
                    bc[:, 0:w],
                )

            def attention(j, hp, fine_tail=False, pre_tail=None):
                nkb = 4 * (j + 1)
                psyA = psy_p.tile([D + 1, 512], F32, tag="psy", name="psyA")
                psyB = psy_p.tile([D + 1, 512], F32, tag="psy", name="psyB")
                q0_sl = j * 512
                pending = {}

                def y_acc(kb):
                    pq0, ppt = pending.pop(kb)
                    nc.tensor.matmul(
                        psyA[:, pq0:512], va[kb][:, 2 * hp, :],
                        ppt[:, 0, pq0:512],
                        start=(kb == 0), stop=(kb == nkb - 1),
                    )
                    nc.tensor.matmul(
                        psyB[:, pq0:512], va[kb][:, 2 * hp + 1, :],
                        ppt[:, 1, pq0:512],
                        start=(kb == 0), stop=(kb == nkb - 1),
                    )

                for kb in range(nkb):
                    di = kb - 4 * j
                    q0 = 128 * di if di > 0 else 0
                    ksl = slice(kb * 128, (kb + 1) * 128)
                    qsl = slice(q0_sl + q0, q0_sl + 512)
                    # filler BEFORE scores: if scores must wait for the exp
                    # pipeline (psm WAR), the filler runs during the wait
                    # instead of head-blocking behind it
                    yield
                    pss = psm_p.tile([128, 2, 512], F32, tag="psm", name="pss")
                    # two K=64 matmuls, concurrent via PE row tiling
                    nc.tensor.matmul(
                        pss[:, 0, q0:512], kt[hp][0:64, ksl], qt[hp][0:64, qsl],
                        start=True, stop=True,
                    )
                    nc.tensor.matmul(
                        pss[:, 1, q0:512], kt[hp][64:128, ksl],
                        qt[hp][64:128, qsl],
                        start=True, stop=True,
                    )
                    pt = ptp.tile([128, 2, 512], BF16, tag="pt", name="pt")
                    # one exp for both heads: strided AP over the written spans
                    nc.scalar.activation(
                        pt[:, :, q0:512], pss[:, :, q0:512],
                        mybir.ActivationFunctionType.Exp, scale=float(SCALE),
                    )
                    if di >= 0:
                        # zero the upper triangle of the diagonal band, both
                        # heads in one strided op.
                        # NB: must NOT run on gpsimd — mixing native tensor
                        # ops with partition_broadcast there forces a ~5us
                        # microcode library swap per alternation.
                        nc.vector.tensor_mul(
                            pt[:, :, q0:q0 + 128], pt[:, :, q0:q0 + 128],
                            mk[:],
                        )
                    pending[kb] = (q0, pt)
                    yield
                    # y lags two k-tiles so exp+mask latency is fully hidden
                    if kb >= 2:
                        y_acc(kb - 2)
                if nkb >= 2:
                    y_acc(nkb - 2)
                y_acc(nkb - 1)
                if not fine_tail:
                    divide(2 * hp, j, psyA)
                    divide(2 * hp + 1, j, psyB)
                else:
                    # last task: pipeline per-256-col divide chunks straight
                    # into their proj row-tiles so the tail drains overlapped
                    # (128-wide chunks make the gpsimd broadcasts, ~850ns
                    # fixed cost each, dominate)
                    if pre_tail is not None:
                        pre_tail()
                    for s in range(2):
                        divide(2 * hp, j, psyA, s * 256, (s + 1) * 256)
                        divide(2 * hp + 1, j, psyB, s * 256, (s + 1) * 256)
                        proj1_unit(j, 4 * j + 2 * s)
                        proj1_unit(j, 4 * j + 2 * s + 1)

            # cc-split projection for the final j-block: the yt[0] half is
            # computed as filler during the last attention task; the tail
            # only runs the yt[1] matmul and an add
            po0 = {}

            def proj0_unit(t, nb):
                ps = psb_p.tile([128, 512], F32, tag="psb", name="ps0")
                nc.tensor.matmul(
                    ps[:], yt[0][:, t * 128:(t + 1) * 128],
                    wpt[:, 0, nb * 512:(nb + 1) * 512],
                    start=True, stop=True,
                )
                p0 = pp.tile([128, 512], F32, name=f"po0_{t}_{nb}",
                             tag=f"po0_{t}_{nb}")
                nc.vector.tensor_copy(p0[:], ps[:])
                po0[(t, nb)] = p0

            def proj1_unit(j, t):
                for nb in range(2):
                    ps = psb_p.tile([128, 512], F32, tag="psb", name="ps1")
                    nc.tensor.matmul(
                        ps[:], yt[1][:, t * 128:(t + 1) * 128],
                        wpt[:, 1, nb * 512:(nb + 1) * 512],
                        start=True, stop=True,
                    )
                    ot = pop.tile([128, 512], BF16, tag="po", name="po")
                    nc.vector.tensor_add(ot[:], ps[:], po0[(t, nb)][:])
                    nc.sync.dma_start(
                        out[t * 128:(t + 1) * 128,
                            nb * 512:(nb + 1) * 512],
                        ot[:],
                    )

            def proj_unit(j, t, tail=False):
                for nb in range(2):
                    ps = psb_p.tile([128, 512], F32, tag="psb", name="pso")
                    for cc in range(2):
                        nc.tensor.matmul(
                            ps[:],
                            yt[cc][:, t * 128:(t + 1) * 128],
                            wpt[:, cc, nb * 512:(nb + 1) * 512],
                            start=(cc == 0),
                            stop=(cc == 1),
                        )
                    ot = pop.tile([128, 512], BF16, tag="po", name="po")
                    if tail:
                        nc.scalar.copy(ot[:], ps[:])  # ACT idle in the tail
                    else:
                        nc.vector.tensor_copy(ot[:], ps[:])
                    nc.sync.dma_start(
                        out[t * 128:(t + 1) * 128,
                            nb * 512:(nb + 1) * 512],
                        ot[:],
                    )

            # ---------------- interleaved schedule ----------------
            def b_units(ns, q_first=False):
                units = []
                if q_first:
                    # b0: wk/wv DMAs land after wq+x0 — do both q units first
                    for m in range(2):
                        units.append((f"q{m}", lambda ns=ns, m=m: qk_unit(ns, m, "q")))
                    for m in range(2):
                        units.append((f"k{m}", lambda ns=ns, m=m: qk_unit(ns, m, "k")))
                else:
                    for m in range(2):
                        units.append((f"q{m}", lambda ns=ns, m=m: qk_unit(ns, m, "q")))
                        units.append((f"k{m}", lambda ns=ns, m=m: qk_unit(ns, m, "k")))
                for t in range(4 * ns, 4 * ns + 4):
                    units.append((f"v{t}", lambda ns=ns, t=t: v_unit(ns, t)))
                return units

            # emit only q0/k0 of segment 0 eagerly so attention(0,0) can
            # start feeding ACT immediately; the rest flows via fillers
            b0 = b_units(0, q_first=True)
            bq = deque()
            emitted = set()
            for name, u in b0:
                if name in ("q0", "k0"):
                    u()
                    emitted.add((0, name))
                else:
                    bq.append((0, name, u))
            for ns in range(1, NQ):
                for name, u in b_units(ns):
                    bq.append((ns, name, u))
            pq = deque()            # proj units, unlocked per j-block
            nyield = [0]

            def emit_filler(allow_proj=True):
                if bq:
                    ns, name, u = bq.popleft()
                    u()
                    emitted.add((ns, name))
                elif pq and allow_proj:
                    pq.popleft()()

            def drain_pq():
                while bq or pq:
                    emit_filler()

            tasks = [(j, hp) for j in range(NQ) for hp in range(2)]
            last = tasks[-1]
            for j, hp in tasks:
                # att(j,hp) needs only q/k of segment j for its own head
                # pair before starting (k of earlier segments already in);
                # v units are consumed by lagged y's and drain via fillers
                while (j, f"q{hp}") not in emitted or (j, f"k{hp}") not in emitted:
                    emit_filler()
                for _ in attention(j, hp, fine_tail=((j, hp) == last),
                                   pre_tail=drain_pq):
                    nyield[0] += 1
                    # hold proj fillers for the ACT-bound late phase (j>=2),
                    # paced at one per k-tile (every 2nd yield)
                    emit_filler(allow_proj=(j >= 2 and nyield[0] % 2 == 0))
                if hp == 1 and j < NQ - 1:
                    for t in range(4 * j, 4 * j + 4):
                        pq.append(lambda j=j, t=t: proj_unit(j, t))
                if (j, hp) == (last[0], 0):
                    # unlock the cc0 half of the final projection
                    for t in range(4 * last[0], 4 * last[0] + 4):
                        for nb in range(2):
                            pq.append(lambda t=t, nb=nb: proj0_unit(t, nb))
            drain_pq()

    nc.compile()
    return nc


def _causal_mask():
    kk = np.arange(128)[:, None]
    cc = np.arange(128)[None, :]
    return (cc >= kk).astype(np.float32)


def _get_nc():
    if "nc" not in _CACHE:
        _CACHE["nc"] = _build()
    return _CACHE["nc"]


def _run(x, W_qkv, W_proj, trace=False, trace_cores=None):
    import ml_dtypes
    from concourse.bass_utils import run_bass_kernel_spmd

    BF = ml_dtypes.bfloat16
    x = np.asarray(x, dtype=np.float32)
    W_qkv = np.asarray(W_qkv, dtype=np.float32)
    W_proj = np.asarray(W_proj, dtype=np.float32)

    nc = _get_nc()
    m1 = _causal_mask()
    mask = np.ascontiguousarray(np.concatenate([m1, m1], axis=1).astype(BF))
    in_maps = []
    for core in range(8):
        b, hg = core // 4, core % 4
        sl = slice(hg * CS, (hg + 1) * CS)

        def warr(w):  # [K, N] -> [128, (K//128)*N] chunk-major per partition
            return np.ascontiguousarray(
                w.reshape(w.shape[0] // 128, 128, -1)
                .transpose(1, 0, 2).reshape(128, -1).astype(BF)
            )

        in_maps.append({
            "xT": np.ascontiguousarray(x[b].T.astype(BF)),
            "wq": warr(W_qkv[:, sl]),
            "wk": warr(W_qkv[:, C + hg * CS:C + (hg + 1) * CS]),
            "wv": warr(W_qkv[:, 2 * C + hg * CS:2 * C + (hg + 1) * CS]),
            "wp": warr(W_proj[sl, :]),
            "mask": mask,
        })

    res = run_bass_kernel_spmd(
        nc, in_maps, list(range(8)), trace=trace, trace_cores=trace_cores
    )
    outp = np.zeros((B, T, C), dtype=np.float32)
    for core in range(8):
        outp[core // 4] += res.results[core]["out"].astype(np.float32)
    return outp, res


def kernel(x, W_qkv, W_proj):
    outp, _ = _run(x, W_qkv, W_proj)
    return outp
